# revision 1
# baseline (speedup 1.0000x reference)
"""Trainium2 Bass kernel for nn_EulerIntegratorCell (Euler-integration RNN).

Reference computation (per batch row b, sequentially over t = 0..T-1):
    z_t  = concat(x_t, a_{t-1}) @ W1 + b1        (HID=64)
    dk_t = tanh(z_t) @ W2 + b2                   (> 0)
    a_t  = a_{t-1} + C * dk_t ** M               (C=1.5e-11, M=3.8)

Kernel strategy
---------------
The per-step increment g(x, a) = C*dk(x, a)**M is a smooth 2-D function and
the state drifts by at most ~7e-3 over all T=2048 steps.  We therefore:

1. Linearize in `a` around each row's initial state a0 (first-order Taylor —
   validated truncation error ~1e-8), giving a *linear* recurrence on the
   shifted state s_t = a_t - a0:
       s_t = d0(a0) * s_{t-1} + d1(x_t, a0),       s_{-1} = 0
   which maps exactly onto the hardware prefix-scan instruction
   `tensor_tensor_scan` (one instruction per [128 x 2048] tile).

2. Fit g(x, a0) with a single shifted sigmoid in x (host-side, from the
   passed-in weights; grid fit, max abs residual ~1.9e-8 of g~4e-6):
       g(x, a0) ~= T0(a0) + A(a0) * sigmoid(c*x + b(a0))
   b folds into the ACT sigmoid's per-partition *bias* and (A, T0) into one
   two-scalar DVE tensor_scalar (2x mode), so each tile needs only:
   1 ACT sigmoid, 1 DVE tensor_scalar, 1 DVE scan, 1 GPSIMD bias-add
   (+ DMA in/out).  The kernel is DMA-bound (~70us/core model).

3. Data-parallel over 8 NeuronCores: batch 16384 -> 2048 rows per core;
   weights/coefficients replicated; no cross-core communication.  x is
   uploaded as bf16 (validated: no measurable accuracy impact) to halve
   the input DMA traffic; all arithmetic and the output stay fp32.

End-to-end absolute error vs the fp32 reference: ~4.4e-6 (the fp32
reference itself deviates ~2e-6 from fp64 ground truth).
"""

import numpy as np
from contextlib import ExitStack

# Problem constants (hardcoded per harness contract).
C = 1.5e-11
M = 3.8
B, T, HID = 16384, 2048, 64
N_CORES = 8
B_CORE = B // N_CORES          # 2048 rows per core
NT = B_CORE // 128             # 16 batch tiles of 128 rows per core
ADEG = 12                      # degree of the a0-polynomials
NFUNC = 4                      # T0, A, b, Gmean
EXP_C = 2.0                    # sigmoid steepness (global)


def _fit_params(W1, b1, W2, b2):
    """Host-side fit of the sigmoid surrogate (O(grid) work, ~2s).

    Returns PC[4, ADEG+1]: power-basis coefficients in t = 2*a0 - 1 for
    (T0, A, b, Gmean)."""
    from scipy.optimize import minimize_scalar
    W1 = np.asarray(W1, np.float64)
    b1 = np.asarray(b1, np.float64)
    W2 = np.asarray(W2, np.float64).reshape(-1)
    b2v = float(np.asarray(b2).reshape(-1)[0])
    al, be, ga = W1[0], W1[1], b1
    NX, NA = 513, 257
    xs = np.linspace(0.0, 1.0, NX)
    as_ = np.linspace(0.0, 1.0, NA)
    z = xs[:, None, None] * al + as_[None, :, None] * be + ga
    th = np.tanh(z)
    dk = th @ W2 + b2v
    G = C * dk ** M
    GA = C * M * dk ** (M - 1.0) * ((1.0 - th * th) @ (W2 * be))
    sig = lambda v: 1.0 / (1.0 + np.exp(-v))
    T0v = np.empty(NA); Av = np.empty(NA); bv = np.empty(NA)
    for ia in range(NA):
        g = G[:, ia]
        def err_b(b):
            Phi = np.stack([np.ones(NX), sig(EXP_C * xs + b)], 1)
            sol, *_ = np.linalg.lstsq(Phi, g, rcond=None)
            return np.abs(Phi @ sol - g).max()
        res = minimize_scalar(err_b, bounds=(-6.0, 4.0), method="bounded",
                              options={"xatol": 1e-10})
        Phi = np.stack([np.ones(NX), sig(EXP_C * xs + res.x)], 1)
        sol, *_ = np.linalg.lstsq(Phi, g, rcond=None)
        T0v[ia], Av[ia] = sol
        bv[ia] = res.x
    funcs = np.stack([T0v, Av, bv, GA.mean(axis=0)])
    cc = np.polynomial.chebyshev.chebfit(2 * as_ - 1, funcs.T, ADEG)
    rows = []
    for r in range(NFUNC):
        p = np.polynomial.chebyshev.cheb2poly(cc[:, r])
        rows.append(np.pad(p, (0, ADEG + 1 - len(p))))
    return np.array(rows)                                      # [4, ADEG+1]


def _build_nc():
    """Build + compile the per-core Bass program (identical on all cores)."""
    import concourse.tile as tile
    from concourse import bacc, mybir

    D1 = ADEG + 1
    f32 = mybir.dt.float32
    bf16 = mybir.dt.bfloat16
    AF = mybir.ActivationFunctionType
    OP = mybir.AluOpType

    nc = bacc.Bacc("TRN2", target_bir_lowering=False, debug=False)
    xin = nc.dram_tensor("x_sh", [B_CORE, T], bf16, kind="ExternalInput")
    a0in = nc.dram_tensor("a0_sh", [128, NT], f32, kind="ExternalInput")
    ctin = nc.dram_tensor("ctab", [128, NFUNC * D1], f32, kind="ExternalInput")
    out = nc.dram_tensor("out_sh", [B_CORE, T], f32, kind="ExternalOutput")

    with tile.TileContext(nc) as tc, ExitStack() as ctx:
        cpool = ctx.enter_context(tc.tile_pool(name="consts", bufs=1))
        hpool = ctx.enter_context(tc.tile_pool(name="horner", bufs=1))
        xpool = ctx.enter_context(tc.tile_pool(name="x", bufs=8))
        q1pool = ctx.enter_context(tc.tile_pool(name="sg", bufs=6))
        dpool = ctx.enter_context(tc.tile_pool(name="d1", bufs=5))
        spool = ctx.enter_context(tc.tile_pool(name="s", bufs=4))
        opool = ctx.enter_context(tc.tile_pool(name="o", bufs=4))

        # ---- per-row coefficient evaluation (Horner in t = 2*a0 - 1) ----
        a0t = cpool.tile([128, NT], f32)
        nc.sync.dma_start(a0t[:], a0in.ap())
        ctt = cpool.tile([128, NFUNC * D1], f32)
        nc.sync.dma_start(ctt[:], ctin.ap())

        tb = cpool.tile([128, NT], f32)
        nc.vector.tensor_scalar(tb[:], a0t[:], 2.0, -1.0, OP.mult, OP.add)

        SCa = hpool.tile([128, NFUNC * NT], f32)
        SCb = hpool.tile([128, NFUNC * NT], f32)
        v3a = SCa[:].rearrange("p (f c) -> p f c", c=NT)
        v3b = SCb[:].rearrange("p (f c) -> p f c", c=NT)
        tbb = tb[:].unsqueeze(1).broadcast_to((128, NFUNC, NT))

        def ck_bcast(k):
            sl = ctt[:, k * NFUNC:(k + 1) * NFUNC]
            return sl.unsqueeze(2).broadcast_to((128, NFUNC, NT))

        nc.vector.tensor_copy(v3a, ck_bcast(ADEG))
        for k in range(ADEG - 1, -1, -1):
            nc.vector.tensor_mul(v3b, v3a, tbb)
            nc.vector.tensor_add(v3a, v3b, ck_bcast(k))
        SC = SCa
        # d0 = 1 + Gmean: add the 1 at full precision (not in-place: HW
        # rejects same-address read/write).
        nc.vector.tensor_scalar(SCb[:, 3 * NT:4 * NT], SCa[:, 3 * NT:4 * NT],
                                1.0, None, OP.add)
        nc.vector.tensor_copy(SCa[:, 3 * NT:4 * NT], SCb[:, 3 * NT:4 * NT])

        def sc_col(f, i):
            return SC[:, f * NT + i: f * NT + i + 1]

        # ---- main pipeline over the 16 batch tiles ----
        st_tiles = {}

        def emit_tail(j):
            ot = opool.tile([128, T], f32, tag="ot")
            nc.gpsimd.tensor_scalar(ot[:], st_tiles[j][:], a0t[:, j:j + 1],
                                    None, OP.add)
            # Alternate output-DMA between the SP and ACT DGE queues so
            # compute-gated out descriptors never head-of-line block
            # input fetches in a single FIFO.
            eng = nc.sync if j % 2 else nc.scalar
            eng.dma_start(out[j * 128:(j + 1) * 128, :], ot[:])
            del st_tiles[j]

        LAG = 6      # software-pipeline the tail so the GPSIMD add and
                     # out-DMA never head-of-line block upstream stages
        for i in range(NT):
            xt = xpool.tile([128, T], bf16, tag="xt")
            nc.sync.dma_start(xt[:], xin[i * 128:(i + 1) * 128, :])

            sg = q1pool.tile([128, T], f32, tag="sg")
            nc.scalar.activation(sg[:], xt[:], AF.Sigmoid,
                                 bias=sc_col(2, i), scale=float(EXP_C))
            d1 = dpool.tile([128, T], f32, tag="d1")
            nc.vector.tensor_scalar(d1[:], sg[:], sc_col(1, i), sc_col(0, i),
                                    OP.mult, OP.add)

            st = spool.tile([128, T], f32, tag="st")
            nc.vector.tensor_tensor_scan(
                st[:], sc_col(3, i).broadcast_to((128, T)), d1[:], 0.0,
                OP.mult, OP.add)
            st_tiles[i] = st
            if i >= LAG:
                emit_tail(i - LAG)
        for j in sorted(st_tiles):
            emit_tail(j)

    nc.compile()
    return nc


_NC_CACHE = {}


def kernel(x, a0, W1, b1, W2, b2):
    x = np.asarray(x, np.float32)
    a0 = np.asarray(a0, np.float32)
    assert x.shape == (B, T, 1) and a0.shape == (B, 1), (x.shape, a0.shape)

    PC = _fit_params(W1, b1, W2, b2)

    key = "v10"
    if key not in _NC_CACHE:
        _NC_CACHE[key] = _build_nc()
    nc = _NC_CACHE[key]

    # ctab: coefficient table, k-major blocks of NFUNC, replicated over the
    # 128 partitions.
    D1 = ADEG + 1
    row = PC.T.reshape(-1).astype(np.float32)
    ctab = np.broadcast_to(row, (128, NFUNC * D1)).copy()

    import ml_dtypes
    x2 = x[:, :, 0].astype(ml_dtypes.bfloat16)   # upload precision (validated)
    a0v = a0[:, 0]
    in_maps = []
    for cidx in range(N_CORES):
        xs = np.ascontiguousarray(x2[cidx * B_CORE:(cidx + 1) * B_CORE])
        # a0_sh[p, i] = a0 of batch row (core_base + i*128 + p)
        a0s = a0v[cidx * B_CORE:(cidx + 1) * B_CORE].reshape(NT, 128).T.copy()
        in_maps.append({"x_sh": xs, "a0_sh": a0s, "ctab": ctab})

    from concourse.bass_utils import run_bass_kernel_spmd
    # The axon-tunneled device occasionally reports
    # NRT_EXEC_UNIT_UNRECOVERABLE on the first dispatch after a fresh
    # process start; it self-recovers within ~1 min.  Retry defensively.
    import time
    last_exc = None
    for attempt in range(4):
        try:
            res = run_bass_kernel_spmd(nc, in_maps,
                                       core_ids=list(range(N_CORES)))
            break
        except Exception as exc:   # noqa: BLE001 — device-level flake
            last_exc = exc
            time.sleep(20.0 * (attempt + 1))
            if attempt >= 1:
                # Rebuild in case the compiled executable is poisoned.
                _NC_CACHE.pop(key, None)
                _NC_CACHE[key] = nc = _build_nc()
    else:
        raise last_exc
    out = np.concatenate(
        [res.results[cidx]["out_sh"] for cidx in range(N_CORES)], axis=0)
    return np.ascontiguousarray(out[:, :, None].astype(np.float32))

